# revision 27
# baseline (speedup 1.0000x reference)
"""Biased MHSA Trainium2 kernel (8-core SPMD), bf16 compute.

Sharding: core c -> (batch b = c//2, head-group g = c%2); each core computes
attention for 4 of the 8 heads of one batch and the partial output projection
for those heads. Host sums the two head-group partials per batch and adds
bo + bv @ wo.

Performance structure (steady state is ACT-bound: one EXP [128,1024] per
1.09us, 2 per key-tile iteration):
  - All matmuls in bf16 (PSUM accumulation stays fp32).
  - Bias handled as P = exp(S) * exp(bias): the host precomputes exp(bias)
    in bf16; the multiply runs on DVE in 2x bf16 mode from SBUF.
  - PV matmuls run one key-tile behind the scores so the in-order PE queue
    never waits on the exp->mult chain.
  - Softmax normalization never touches DRAM or the DVE's critical stream:
    1/r via reciprocal_approx_fast on the PSUM aug-row, pu eviction on
    GpSimd, r broadcast across the 64 feature partitions by a tiny f32r PE
    matmul into the PSUM slot pu just vacated, A^T = U^T * bc on DVE. The
    PE/DVE halves of that chain are deferred into the NEXT pass's kt loop
    (kt==1) so pass boundaries stay pipelined; pu tiles are allocated
    lazily at kt==1 to keep the PSUM tag rotation pu(p)->bc(p)->pu(p+1).
  - O-projection tiles for chunk q2-1 are spread through the next pass's
    kt loop (kt 3/6/9/12), evicted on GpSimd, so the psum_mm rotation and
    the DVE stream never see a burst.
  - Startup: every input DMA rides the sync queue in priority order
    (x, wk, wq, wv, wo, biases, then exp(bias) tiles) so projections are
    never starved by the 8MB bias stream; 8 dummy matmuls pre-warm the HAM
    clock gate and a dummy exp preloads the ACT Exp table.
"""

import sys

if "/opt/trn_rl_repo" not in sys.path:
    sys.path.insert(0, "/opt/trn_rl_repo")

from contextlib import ExitStack

import numpy as np
import ml_dtypes

import concourse.bass as bass
from concourse import bacc
import concourse.tile as tile
from concourse import mybir
from concourse.bass_utils import run_bass_kernel_spmd

B, N, D = 4, 2048, 512
H, DH = 8, 64
HG = 4  # heads per core
GD = HG * DH  # 256 features per core
P = 128
QQ = 512  # matmul moving-dim slice
QW = 1024  # q processed in chunks of 1024
NQW = N // QW  # 2
NSL = QW // QQ  # 2
PVLAG = 3  # PV runs 3 key-tiles behind the scores
NKT = N // P  # 16 key tiles
KC = D // P  # 4 contraction chunks for projections
NN = N // QQ  # 4 token chunks for projections
F32 = mybir.dt.float32
F32R = mybir.dt.float32r
BF16 = mybir.dt.bfloat16
BF_NP = ml_dtypes.bfloat16


def build_program():
    nc = bacc.Bacc("TRN2", target_bir_lowering=False)
    xT = nc.dram_tensor("xT", [D, N], BF16, kind="ExternalInput")
    expbT = nc.dram_tensor("expbT", [N, N], BF16, kind="ExternalInput")
    wq = nc.dram_tensor("wq", [D, GD], BF16, kind="ExternalInput")
    wk = nc.dram_tensor("wk", [D, GD], BF16, kind="ExternalInput")
    wv = nc.dram_tensor("wv", [D, GD], BF16, kind="ExternalInput")
    wo = nc.dram_tensor("wo", [GD, D], BF16, kind="ExternalInput")
    bq = nc.dram_tensor("bq", [GD], F32, kind="ExternalInput")
    bk = nc.dram_tensor("bk", [GD], F32, kind="ExternalInput")
    out = nc.dram_tensor("out", [N, D], F32, kind="ExternalOutput")

    with tile.TileContext(nc) as tc, ExitStack() as ctx:
        const = ctx.enter_context(tc.tile_pool(name="const", bufs=1))
        big = ctx.enter_context(tc.tile_pool(name="big", bufs=1))
        p_pool = ctx.enter_context(tc.tile_pool(name="probs", bufs=8))
        small = ctx.enter_context(tc.tile_pool(name="small", bufs=2))
        o_pool = ctx.enter_context(tc.tile_pool(name="outp", bufs=6))
        psum_mm = ctx.enter_context(tc.tile_pool(name="psum_mm", bufs=2, space="PSUM"))
        psum_u = ctx.enter_context(tc.tile_pool(name="psum_u", bufs=1, space="PSUM"))
        dram_p = ctx.enter_context(tc.tile_pool(name="dram_p", bufs=2, space="DRAM"))

        # ---- HAM warm-up + ACT Exp-table preload (runs during input DMAs) --
        warm = const.tile([P, QQ], BF16)
        nc.vector.memset(warm, 0.0)
        expd = const.tile([P, 2], BF16)
        nc.scalar.activation(expd, warm[:, 0:2], mybir.ActivationFunctionType.Exp)
        wps = psum_mm.tile([P, QW], F32, tag="mm", name="warmps")
        for _ in range(40):  # bridge the input-DMA wait so HAM stays warm
            nc.tensor.matmul(wps[:, 0:QQ], warm[:, 0:P], warm, start=True, stop=True)

        # ---- load inputs: one priority-ordered queue (x+weights before the
        # 8MB exp(bias) stream; the DGE spreads descriptors over all 16 HW
        # engines regardless of which queue issues) ----------------------
        bq_s = const.tile([P, 2], F32)
        nc.sync.dma_start(out=bq_s, in_=bq.rearrange("(fc p) -> p fc", p=P))
        bk_s = const.tile([P, 2], F32)
        nc.sync.dma_start(out=bk_s, in_=bk.rearrange("(fc p) -> p fc", p=P))
        bqs = const.tile([P, 2], F32)  # bq * 0.125 (scale folded into Q)
        nc.vector.tensor_scalar_mul(bqs, bq_s, 0.125)
        wq_s = const.tile([P, KC, GD], BF16)
        wk_s = const.tile([P, KC, GD], BF16)
        wv_s = const.tile([P, KC, GD], BF16)
        xT_s = big.tile([P, KC, N], BF16)  # x^T as [128, kc, tok]
        for kc in range(KC):
            nc.sync.dma_start(out=wk_s[:, kc, :], in_=wk[kc * P : (kc + 1) * P, :])
        for nn in range(NN):
            for kc in range(KC):
                nc.sync.dma_start(
                    out=xT_s[:, kc, nn * QQ : (nn + 1) * QQ],
                    in_=xT[kc * P : (kc + 1) * P, nn * QQ : (nn + 1) * QQ],
                )
        for kc in range(KC):
            nc.sync.dma_start(out=wq_s[:, kc, :], in_=wq[kc * P : (kc + 1) * P, :])
        for kc in range(KC):
            nc.sync.dma_start(out=wv_s[:, kc, :], in_=wv[kc * P : (kc + 1) * P, :])
        wo_s = const.tile([P, 2, D], BF16)  # head-pair wo rows: [128, hp, 512]
        nc.sync.dma_start(out=wo_s, in_=wo.rearrange("(g p) d -> p g d", p=P))
        # exp(bias)^T tiles, bf16, after all compute-critical inputs
        ebt = big.tile([P, NQW, NKT, QW], BF16)
        for q2 in range(NQW):
            for kt in range(NKT):
                nc.sync.dma_start(
                    out=ebt[:, q2, kt, :],
                    in_=expbT[kt * P : (kt + 1) * P, q2 * QW : (q2 + 1) * QW],
                )

        # ---- projections (K first: scores need all of K; then Q, then V) --
        # Q^T, K^T: [128, fc, tok] (feature on partitions; head-pair fc has
        # head 2fc on partitions 0..63 and head 2fc+1 on 64..127)
        qT = big.tile([P, 2, N], BF16)
        kT = big.tile([P, 2, N], BF16)
        vaug = big.tile([P, HG, NKT, DH + 1], BF16)  # [tok, h, kt, 64 V | 1]
        nc.vector.memset(vaug[:, :, :, DH : DH + 1], 1.0)
        for w_s, dst, b_ap, scale in (
            (wk_s, kT, bk_s, 1.0),
            (wq_s, qT, bqs, 0.125),
        ):
            for nn in range(NN):
                for fc in range(2):
                    ps = psum_mm.tile([P, QW], F32, tag="mm", name="ps_p")
                    for kc in range(KC):
                        nc.tensor.matmul(
                            ps[:, 0:QQ],
                            (w_s[:, kc, fc * P : (fc + 1) * P]),
                            (xT_s[:, kc, nn * QQ : (nn + 1) * QQ]),
                            start=(kc == 0),
                            stop=(kc == KC - 1),
                        )
                    # (x@w + b) * scale  ==  psum*scale + b*scale
                    nc.vector.tensor_scalar(
                        dst[:, fc, nn * QQ : (nn + 1) * QQ],
                        ps[:, 0:QQ],
                        scale,
                        b_ap[:, fc : fc + 1],
                        op0=mybir.AluOpType.mult,
                        op1=mybir.AluOpType.add,
                    )
        for kt in range(NKT):  # V natural layout (bv folded on host)
            ps = psum_mm.tile([P, QW], F32, tag="mm", name="ps_v")
            for kc in range(KC):
                nc.tensor.matmul(
                    ps[:, 0:GD],
                    (xT_s[:, kc, kt * P : (kt + 1) * P]),
                    (wv_s[:, kc, :]),
                    start=(kc == 0),
                    stop=(kc == KC - 1),
                )
            nc.vector.tensor_copy(
                vaug[:, :, kt, 0:DH],
                ps[:, 0:GD].rearrange("p (h d) -> p h d", h=HG),
            )

        # ---- attention ----
        # A^T (normalized attn out), head pair stacked on 128 partitions:
        # head 2hp+i at partitions i*64..(i+1)*64 of chunk hp. One tile per
        # q chunk so the O-proj tiles for chunk q2-1 carry no (coarse-grain)
        # dependency on chunk q2's normalize writes.
        aTs = [big.tile([P, 2, QW], BF16, name=f"aT{j}") for j in range(NQW)]

        def norm_stage_a(pu_h):
            # Evict U_aug^T (releases pu's PSUM slot) and launch the r-row
            # partition-fold DMA: [1,1024] row -> DRAM -> [128,8] so the
            # reciprocal is a ~0.2us op instead of a 6.5us one. The DMA
            # latency hides under the next pass's first key-tiles.
            uT = small.tile([DH + 1, QW], F32, tag="uT", name="uT")
            nc.vector.tensor_copy(uT, pu_h)
            r_d = dram_p.tile([QW], F32, tag="rd", name="r_d")
            nc.gpsimd.dma_start(out=r_d[:], in_=uT[DH : DH + 1, :])
            r128 = small.tile([P, QW // P], F32, tag="r128", name="r128")
            nc.gpsimd.dma_start(out=r128, in_=r_d[:].rearrange("(f p) -> p f", p=P))
            return uT, r128

        def norm_stage_a2(r128):
            # 1/r on the folded layout, then unfold + broadcast to the 64
            # feature partitions (stride-0 DMA read from DRAM).
            nc.vector.reciprocal(r128, r128)
            rd2 = dram_p.tile([QW], F32, tag="rd2", name="rd2")
            nc.gpsimd.dma_start(out=rd2[:].rearrange("(f p) -> p f", p=P), in_=r128)
            bc = small.tile([DH, QW], F32, tag="bc", name="bc")
            rap = rd2[:]
            nc.gpsimd.dma_start(
                out=bc,
                in_=bass.AP(
                    tensor=rap.tensor, offset=rap.offset,
                    ap=[[0, DH]] + list(rap.ap),
                ),
            )
            return bc

        def norm_stage_b(q2, hp, i, uT, bc):
            nc.vector.tensor_tensor(
                aTs[q2][i * DH : (i + 1) * DH, hp, :],
                uT[0:DH, :],
                bc,
                op=mybir.AluOpType.mult,
            )

        def oproj_tile(t):
            # O[tok, 512] = sum_hp A_pair^T.T @ wo_pair for one token tile
            aT_t = aTs[t // (QW // P)]
            tl = t % (QW // P)
            ps = psum_mm.tile([P, QW], F32, tag="mm", name="ps_o")
            for hp in range(2):
                nc.tensor.matmul(
                    ps[:, 0:D],
                    aT_t[:, hp, tl * P : (tl + 1) * P],
                    wo_s[:, hp, :],
                    start=(hp == 0),
                    stop=(hp == 1),
                )
            ob = o_pool.tile([P, D], F32, name="ob")
            nc.vector.tensor_copy(ob, ps[:, 0:D])
            nc.sync.dma_start(out=out[t * P : (t + 1) * P, :], in_=ob)

        pending_norm = None  # deferred stage-B args from the previous pass
        for q2 in range(NQW):
            for hp in range(2):  # head pair: heads (2hp, 2hp+1) live in fc=hp
                pu = None
                # O-proj tiles for chunk q2-1 spread through this pass
                otiles = (
                    [(q2 - 1) * (QW // P) + hp * 4 + t for t in range(4)]
                    if q2 > 0
                    else []
                )

                def emit_pv(j, sps):
                    for i in range(2):
                        for sl in range(NSL):
                            ssl = slice(sl * QQ, (sl + 1) * QQ)
                            nc.tensor.matmul(
                                pu[i][:, ssl],
                                vaug[:, 2 * hp + i, j, :],
                                sps[i][:, ssl],
                                start=(j == 0),
                                stop=(j == NKT - 1),
                            )

                sp_hist = []
                for kt in range(NKT):
                    # scores for both heads; sl-outer/i-inner so consecutive
                    # instructions pair up on disjoint PE row groups
                    ps = [
                        psum_mm.tile([P, QW], F32, tag="mm", name=f"ps{i}")
                        for i in range(2)
                    ]
                    for sl in range(NSL):
                        ssl = slice(sl * QQ, (sl + 1) * QQ)
                        for i in range(2):
                            ho = i * DH
                            nc.tensor.matmul(
                                ps[i][:, ssl],
                                kT[ho : ho + DH, hp, kt * P : (kt + 1) * P],
                                qT[ho : ho + DH, hp, q2 * QW + sl * QQ :
                                   q2 * QW + (sl + 1) * QQ],
                                start=True,
                                stop=True,
                            )
                    sp_cur = []
                    for i in range(2):
                        sp = p_pool.tile([P, QW], BF16, tag="sp", name="sp")
                        nc.scalar.activation(
                            sp, ps[i], mybir.ActivationFunctionType.Exp
                        )
                        # P = exp(S) * exp(bias): 2x bf16 DVE mode, in place
                        nc.vector.tensor_tensor(
                            sp, sp, ebt[:, q2, kt, :], op=mybir.AluOpType.mult
                        )
                        sp_cur.append(sp)
                    if kt == 2:
                        # this pass's accumulators (pu(p-1) freed at stage
                        # A's eviction, long before this)
                        pu = [
                            psum_u.tile([DH + 1, QW], F32, tag=f"u{i}", name=f"pu{i}")
                            for i in range(2)
                        ]
                    if kt == 3 and pending_norm:
                        # previous pass's 1/r + broadcast (r-fold DMA landed)
                        for st in pending_norm:
                            st.append(norm_stage_a2(st[4]))
                    if kt in (8, 10) and pending_norm and len(pending_norm[0]) == 6:
                        # previous pass's A^T = U^T * (1/r): one head per kt,
                        # placed after the ~15us broadcast-DMA chain has
                        # landed so the DVE stream never head-of-line blocks
                        st = pending_norm.pop(0)
                        norm_stage_b(st[0], st[1], st[2], st[3], st[5])
                        if not pending_norm:
                            pending_norm = None
                    if kt >= 11 and otiles:
                        oproj_tile(otiles.pop(0))
                    if kt >= PVLAG:
                        emit_pv(kt - PVLAG, sp_hist[0])
                        sp_hist.pop(0)
                    sp_hist.append(sp_cur)
                for j in range(NKT - PVLAG, NKT):
                    emit_pv(j, sp_hist[0])
                    sp_hist.pop(0)
                pending_norm = []
                for i in range(2):
                    uT, r128 = norm_stage_a(pu[i])
                    pending_norm.append([q2, hp, i, uT, r128])
        # final pass: nothing left to overlap with; emit the rest directly
        for st in pending_norm:
            st.append(norm_stage_a2(st[4]))
        for st in pending_norm:
            norm_stage_b(st[0], st[1], st[2], st[3], st[5])
        for t in range((NQW - 1) * QW // P, NQW * QW // P):
            oproj_tile(t)

    nc.compile()
    return nc


_NC = None


def _get_nc():
    global _NC
    if _NC is None:
        _NC = build_program()
    return _NC


def make_in_maps(x, attn_bias, wq, bq, wk, bk, wv, bv, wo, bo):
    x = np.asarray(x, np.float32)
    attn_bias = np.asarray(attn_bias, np.float32)
    expbT_b = [
        np.ascontiguousarray(np.exp(attn_bias[b, 0]).T.astype(BF_NP))
        for b in range(B)
    ]
    xT_b = [np.ascontiguousarray(x[b].T.astype(BF_NP)) for b in range(B)]
    wq = np.asarray(wq, np.float32)
    wk = np.asarray(wk, np.float32)
    wv = np.asarray(wv, np.float32)
    wo = np.asarray(wo, np.float32)
    in_maps = []
    for c in range(8):
        b, g = c // 2, c % 2
        sl = slice(g * GD, (g + 1) * GD)
        in_maps.append(
            {
                "xT": xT_b[b],
                "expbT": expbT_b[b],
                "wq": np.ascontiguousarray(wq[:, sl].astype(BF_NP)),
                "wk": np.ascontiguousarray(wk[:, sl].astype(BF_NP)),
                "wv": np.ascontiguousarray(wv[:, sl].astype(BF_NP)),
                "wo": np.ascontiguousarray(wo[sl, :].astype(BF_NP)),
                "bq": np.ascontiguousarray(np.asarray(bq, np.float32)[sl]),
                "bk": np.ascontiguousarray(np.asarray(bk, np.float32)[sl]),
            }
        )
    return in_maps


def gather_output(results, bo, bv, wo):
    bo = np.asarray(bo, np.float32)
    row = bo + np.asarray(bv, np.float32) @ np.asarray(wo, np.float32)
    out = np.empty((B, N, D), np.float32)
    for b in range(B):
        out[b] = results[2 * b]["out"] + results[2 * b + 1]["out"] + row[None, :]
    return out


def kernel(x, attn_bias, wq, bq, wk, bk, wv, bv, wo, bo, _trace=False):
    nc = _get_nc()
    in_maps = make_in_maps(x, attn_bias, wq, bq, wk, bk, wv, bv, wo, bo)
    res = run_bass_kernel_spmd(nc, in_maps, core_ids=list(range(8)), trace=_trace)
    out = gather_output(res.results, bo, bv, wo)
    if _trace:
        kernel.last_results = res
    return out


# revision 29
# speedup vs baseline: 1.0345x; 1.0345x over previous
"""Biased MHSA Trainium2 kernel (8-core SPMD), bf16 compute.

Sharding: core c -> (batch b = c//2, head-group g = c%2); each core computes
attention for 4 of the 8 heads of one batch and the partial output projection
for those heads. Host sums the two head-group partials per batch and adds
bo + bv @ wo.

Performance structure (steady state is ACT-bound: one EXP [128,1024] per
1.09us, 2 per key-tile iteration):
  - All matmuls in bf16 (PSUM accumulation stays fp32).
  - Bias handled as P = exp(S) * exp(bias): the host precomputes exp(bias)
    in bf16; the multiply runs on DVE in 2x bf16 mode from SBUF.
  - PV matmuls run one key-tile behind the scores so the in-order PE queue
    never waits on the exp->mult chain.
  - Softmax normalization never touches DRAM or the DVE's critical stream:
    1/r via reciprocal_approx_fast on the PSUM aug-row, pu eviction on
    GpSimd, r broadcast across the 64 feature partitions by a tiny f32r PE
    matmul into the PSUM slot pu just vacated, A^T = U^T * bc on DVE. The
    PE/DVE halves of that chain are deferred into the NEXT pass's kt loop
    (kt==1) so pass boundaries stay pipelined; pu tiles are allocated
    lazily at kt==1 to keep the PSUM tag rotation pu(p)->bc(p)->pu(p+1).
  - O-projection tiles for chunk q2-1 are spread through the next pass's
    kt loop (kt 3/6/9/12), evicted on GpSimd, so the psum_mm rotation and
    the DVE stream never see a burst.
  - Startup: every input DMA rides the sync queue in priority order
    (x, wk, wq, wv, wo, biases, then exp(bias) tiles) so projections are
    never starved by the 8MB bias stream; 8 dummy matmuls pre-warm the HAM
    clock gate and a dummy exp preloads the ACT Exp table.
"""

import sys

if "/opt/trn_rl_repo" not in sys.path:
    sys.path.insert(0, "/opt/trn_rl_repo")

from contextlib import ExitStack

import numpy as np
import ml_dtypes

import concourse.bass as bass
from concourse import bacc
import concourse.tile as tile
from concourse import mybir
from concourse.bass_utils import run_bass_kernel_spmd

B, N, D = 4, 2048, 512
H, DH = 8, 64
HG = 4  # heads per core
GD = HG * DH  # 256 features per core
P = 128
QQ = 512  # matmul moving-dim slice
QW = 1024  # q processed in chunks of 1024
NQW = N // QW  # 2
NSL = QW // QQ  # 2
PVLAG = 3  # PV runs 3 key-tiles behind the scores
NKT = N // P  # 16 key tiles
KC = D // P  # 4 contraction chunks for projections
NN = N // QQ  # 4 token chunks for projections
F32 = mybir.dt.float32
F32R = mybir.dt.float32r
BF16 = mybir.dt.bfloat16
BF_NP = ml_dtypes.bfloat16


def build_program():
    nc = bacc.Bacc("TRN2", target_bir_lowering=False)
    xT = nc.dram_tensor("xT", [D, N], BF16, kind="ExternalInput")
    expbT = nc.dram_tensor("expbT", [N, N], BF16, kind="ExternalInput")
    wq = nc.dram_tensor("wq", [D, GD], BF16, kind="ExternalInput")
    wk = nc.dram_tensor("wk", [D, GD], BF16, kind="ExternalInput")
    wv = nc.dram_tensor("wv", [D, GD], BF16, kind="ExternalInput")
    wo = nc.dram_tensor("wo", [GD, D], BF16, kind="ExternalInput")
    bq = nc.dram_tensor("bq", [GD], F32, kind="ExternalInput")
    bk = nc.dram_tensor("bk", [GD], F32, kind="ExternalInput")
    out = nc.dram_tensor("out", [N, D], F32, kind="ExternalOutput")

    with tile.TileContext(nc) as tc, ExitStack() as ctx:
        const = ctx.enter_context(tc.tile_pool(name="const", bufs=1))
        big = ctx.enter_context(tc.tile_pool(name="big", bufs=1))
        p_pool = ctx.enter_context(tc.tile_pool(name="probs", bufs=8))
        small = ctx.enter_context(tc.tile_pool(name="small", bufs=2))
        o_pool = ctx.enter_context(tc.tile_pool(name="outp", bufs=6))
        psum_mm = ctx.enter_context(tc.tile_pool(name="psum_mm", bufs=2, space="PSUM"))
        psum_u = ctx.enter_context(tc.tile_pool(name="psum_u", bufs=1, space="PSUM"))
        dram_p = ctx.enter_context(tc.tile_pool(name="dram_p", bufs=2, space="DRAM"))

        # ---- HAM warm-up + ACT Exp-table preload (runs during input DMAs) --
        warm = const.tile([P, QQ], BF16)
        nc.vector.memset(warm, 0.0)
        expd = const.tile([P, 2], BF16)
        nc.scalar.activation(expd, warm[:, 0:2], mybir.ActivationFunctionType.Exp)
        wps = psum_mm.tile([P, QW], F32, tag="mm", name="warmps")
        for _ in range(40):  # bridge the input-DMA wait so HAM stays warm
            nc.tensor.matmul(wps[:, 0:QQ], warm[:, 0:P], warm, start=True, stop=True)

        # ---- load inputs: one priority-ordered queue (x+weights before the
        # 8MB exp(bias) stream; the DGE spreads descriptors over all 16 HW
        # engines regardless of which queue issues) ----------------------
        bq_s = const.tile([P, 2], F32)
        nc.sync.dma_start(out=bq_s, in_=bq.rearrange("(fc p) -> p fc", p=P))
        bk_s = const.tile([P, 2], F32)
        nc.sync.dma_start(out=bk_s, in_=bk.rearrange("(fc p) -> p fc", p=P))
        bqs = const.tile([P, 2], F32)  # bq * 0.125 (scale folded into Q)
        nc.vector.tensor_scalar_mul(bqs, bq_s, 0.125)
        wq_s = const.tile([P, KC, GD], BF16)
        wk_s = const.tile([P, KC, GD], BF16)
        wv_s = const.tile([P, KC, GD], BF16)
        xT_s = big.tile([P, KC, N], BF16)  # x^T as [128, kc, tok]
        for kc in range(KC):
            nc.sync.dma_start(out=wk_s[:, kc, :], in_=wk[kc * P : (kc + 1) * P, :])
        for nn in range(NN):
            for kc in range(KC):
                nc.sync.dma_start(
                    out=xT_s[:, kc, nn * QQ : (nn + 1) * QQ],
                    in_=xT[kc * P : (kc + 1) * P, nn * QQ : (nn + 1) * QQ],
                )
        for kc in range(KC):
            nc.sync.dma_start(out=wq_s[:, kc, :], in_=wq[kc * P : (kc + 1) * P, :])
        for kc in range(KC):
            nc.sync.dma_start(out=wv_s[:, kc, :], in_=wv[kc * P : (kc + 1) * P, :])
        wo_s = const.tile([P, 2, D], BF16)  # head-pair wo rows: [128, hp, 512]
        nc.sync.dma_start(out=wo_s, in_=wo.rearrange("(g p) d -> p g d", p=P))
        # exp(bias)^T tiles, bf16, after all compute-critical inputs
        ebt = big.tile([P, NQW, NKT, QW], BF16)
        for q2 in range(NQW):
            for kt in range(NKT):
                nc.sync.dma_start(
                    out=ebt[:, q2, kt, :],
                    in_=expbT[kt * P : (kt + 1) * P, q2 * QW : (q2 + 1) * QW],
                )

        # ---- projections (K first: scores need all of K; then Q, then V) --
        # Q^T, K^T: [128, fc, tok] (feature on partitions; head-pair fc has
        # head 2fc on partitions 0..63 and head 2fc+1 on 64..127)
        qT = big.tile([P, 2, N], BF16)
        kT = big.tile([P, 2, N], BF16)
        vaug = big.tile([P, HG, NKT, DH + 1], BF16)  # [tok, h, kt, 64 V | 1]
        nc.vector.memset(vaug[:, :, :, DH : DH + 1], 1.0)
        for w_s, dst, b_ap, scale in (
            (wk_s, kT, bk_s, 1.0),
            (wq_s, qT, bqs, 0.125),
        ):
            for nn in range(NN):
                for fc in range(2):
                    ps = psum_mm.tile([P, QW], F32, tag="mm", name="ps_p")
                    for kc in range(KC):
                        nc.tensor.matmul(
                            ps[:, 0:QQ],
                            (w_s[:, kc, fc * P : (fc + 1) * P]),
                            (xT_s[:, kc, nn * QQ : (nn + 1) * QQ]),
                            start=(kc == 0),
                            stop=(kc == KC - 1),
                        )
                    # (x@w + b) * scale  ==  psum*scale + b*scale
                    nc.vector.tensor_scalar(
                        dst[:, fc, nn * QQ : (nn + 1) * QQ],
                        ps[:, 0:QQ],
                        scale,
                        b_ap[:, fc : fc + 1],
                        op0=mybir.AluOpType.mult,
                        op1=mybir.AluOpType.add,
                    )
        for kt in range(NKT):  # V natural layout (bv folded on host)
            ps = psum_mm.tile([P, QW], F32, tag="mm", name="ps_v")
            for kc in range(KC):
                nc.tensor.matmul(
                    ps[:, 0:GD],
                    (xT_s[:, kc, kt * P : (kt + 1) * P]),
                    (wv_s[:, kc, :]),
                    start=(kc == 0),
                    stop=(kc == KC - 1),
                )
            nc.vector.tensor_copy(
                vaug[:, :, kt, 0:DH],
                ps[:, 0:GD].rearrange("p (h d) -> p h d", h=HG),
            )

        # ---- attention ----
        # A^T (normalized attn out), head pair stacked on 128 partitions:
        # head 2hp+i at partitions i*64..(i+1)*64 of chunk hp. One tile per
        # q chunk so the O-proj tiles for chunk q2-1 carry no (coarse-grain)
        # dependency on chunk q2's normalize writes.
        aTs = [big.tile([P, 2, QW], BF16, name=f"aT{j}") for j in range(NQW)]

        def norm_stage_a(pu_h):
            # Evict U_aug^T (releases pu's PSUM slot) and launch the r-row
            # partition-fold DMA: [1,1024] row -> DRAM -> [128,8] so the
            # reciprocal is a ~0.2us op instead of a 6.5us one. The DMA
            # latency hides under the next pass's first key-tiles.
            uT = small.tile([DH + 1, QW], F32, tag="uT", name="uT")
            nc.vector.tensor_copy(uT, pu_h)
            r_d = dram_p.tile([QW], F32, tag="rd", name="r_d")
            nc.gpsimd.dma_start(out=r_d[:], in_=uT[DH : DH + 1, :])
            r128 = small.tile([P, QW // P], F32, tag="r128", name="r128")
            nc.gpsimd.dma_start(out=r128, in_=r_d[:].rearrange("(f p) -> p f", p=P))
            return uT, r128

        def norm_stage_a2(r128):
            # 1/r on the folded layout, then unfold + broadcast to the 64
            # feature partitions (stride-0 DMA read from DRAM).
            nc.vector.reciprocal(r128, r128)
            rd2 = dram_p.tile([QW], F32, tag="rd2", name="rd2")
            nc.gpsimd.dma_start(out=rd2[:].rearrange("(f p) -> p f", p=P), in_=r128)
            bc = small.tile([DH, QW], F32, tag="bc", name="bc")
            rap = rd2[:]
            nc.gpsimd.dma_start(
                out=bc,
                in_=bass.AP(
                    tensor=rap.tensor, offset=rap.offset,
                    ap=[[0, DH]] + list(rap.ap),
                ),
            )
            return bc

        def norm_stage_b(q2, hp, i, uT, bc):
            nc.vector.tensor_tensor(
                aTs[q2][i * DH : (i + 1) * DH, hp, :],
                uT[0:DH, :],
                bc,
                op=mybir.AluOpType.mult,
            )

        def oproj_tile(t):
            # O[tok, 512] = sum_hp A_pair^T.T @ wo_pair for one token tile
            aT_t = aTs[t // (QW // P)]
            tl = t % (QW // P)
            ps = psum_mm.tile([P, QW], F32, tag="mm", name="ps_o")
            for hp in range(2):
                nc.tensor.matmul(
                    ps[:, 0:D],
                    aT_t[:, hp, tl * P : (tl + 1) * P],
                    wo_s[:, hp, :],
                    start=(hp == 0),
                    stop=(hp == 1),
                )
            ob = o_pool.tile([P, D], F32, name="ob")
            nc.vector.tensor_copy(ob, ps[:, 0:D])
            nc.sync.dma_start(out=out[t * P : (t + 1) * P, :], in_=ob)

        pending_norm = None  # deferred stage-B args from the previous pass
        for q2 in range(NQW):
            for hp in range(2):  # head pair: heads (2hp, 2hp+1) live in fc=hp
                pu = None
                # O-proj tiles for chunk q2-1 spread through this pass
                otiles = (
                    [(q2 - 1) * (QW // P) + hp * 4 + t for t in range(4)]
                    if q2 > 0
                    else []
                )

                def emit_pv(j, sps):
                    for i in range(2):
                        for sl in range(NSL):
                            ssl = slice(sl * QQ, (sl + 1) * QQ)
                            nc.tensor.matmul(
                                pu[i][:, ssl],
                                vaug[:, 2 * hp + i, j, :],
                                sps[i][:, ssl],
                                start=(j == 0),
                                stop=(j == NKT - 1),
                            )

                sp_hist = []
                for kt in range(NKT):
                    # scores for both heads; sl-outer/i-inner so consecutive
                    # instructions pair up on disjoint PE row groups
                    ps = [
                        psum_mm.tile([P, QW], F32, tag="mm", name=f"ps{i}")
                        for i in range(2)
                    ]
                    for sl in range(NSL):
                        ssl = slice(sl * QQ, (sl + 1) * QQ)
                        for i in range(2):
                            ho = i * DH
                            nc.tensor.matmul(
                                ps[i][:, ssl],
                                kT[ho : ho + DH, hp, kt * P : (kt + 1) * P],
                                qT[ho : ho + DH, hp, q2 * QW + sl * QQ :
                                   q2 * QW + (sl + 1) * QQ],
                                start=True,
                                stop=True,
                            )
                    sp_cur = []
                    for i in range(2):
                        sp = p_pool.tile([P, QW], BF16, tag="sp", name="sp")
                        nc.scalar.activation(
                            sp, ps[i], mybir.ActivationFunctionType.Exp
                        )
                        # P = exp(S) * exp(bias): 2x bf16 DVE mode, in place
                        nc.vector.tensor_tensor(
                            sp, sp, ebt[:, q2, kt, :], op=mybir.AluOpType.mult
                        )
                        sp_cur.append(sp)
                    if kt == 2:
                        # this pass's accumulators (pu(p-1) freed at stage
                        # A's eviction, long before this)
                        pu = [
                            psum_u.tile([DH + 1, QW], F32, tag=f"u{i}", name=f"pu{i}")
                            for i in range(2)
                        ]
                    if kt == 3 and pending_norm:
                        # previous pass's 1/r + broadcast (r-fold DMA landed)
                        for st in pending_norm:
                            st.append(norm_stage_a2(st[4]))
                    if kt in (7, 9) and pending_norm and len(pending_norm[0]) == 6:
                        # previous pass's A^T = U^T * (1/r): one head per kt,
                        # placed after the ~15us broadcast-DMA chain has
                        # landed so the DVE stream never head-of-line blocks
                        st = pending_norm.pop(0)
                        norm_stage_b(st[0], st[1], st[2], st[3], st[5])
                        if not pending_norm:
                            pending_norm = None
                    if kt in (11, 13, 15) and otiles:
                        oproj_tile(otiles.pop(0))
                    if kt >= PVLAG:
                        emit_pv(kt - PVLAG, sp_hist[0])
                        sp_hist.pop(0)
                    sp_hist.append(sp_cur)
                for j in range(NKT - PVLAG, NKT):
                    emit_pv(j, sp_hist[0])
                    sp_hist.pop(0)
                while otiles:
                    oproj_tile(otiles.pop(0))
                pending_norm = []
                for i in range(2):
                    uT, r128 = norm_stage_a(pu[i])
                    pending_norm.append([q2, hp, i, uT, r128])
        # final pass: nothing left to overlap with; emit the rest directly
        for st in pending_norm:
            st.append(norm_stage_a2(st[4]))
        for st in pending_norm:
            norm_stage_b(st[0], st[1], st[2], st[3], st[5])
        for t in range((NQW - 1) * QW // P, NQW * QW // P):
            oproj_tile(t)

    nc.compile()
    return nc


_NC = None


def _get_nc():
    global _NC
    if _NC is None:
        _NC = build_program()
    return _NC


def make_in_maps(x, attn_bias, wq, bq, wk, bk, wv, bv, wo, bo):
    x = np.asarray(x, np.float32)
    attn_bias = np.asarray(attn_bias, np.float32)
    expbT_b = [
        np.ascontiguousarray(np.exp(attn_bias[b, 0]).T.astype(BF_NP))
        for b in range(B)
    ]
    xT_b = [np.ascontiguousarray(x[b].T.astype(BF_NP)) for b in range(B)]
    wq = np.asarray(wq, np.float32)
    wk = np.asarray(wk, np.float32)
    wv = np.asarray(wv, np.float32)
    wo = np.asarray(wo, np.float32)
    in_maps = []
    for c in range(8):
        b, g = c // 2, c % 2
        sl = slice(g * GD, (g + 1) * GD)
        in_maps.append(
            {
                "xT": xT_b[b],
                "expbT": expbT_b[b],
                "wq": np.ascontiguousarray(wq[:, sl].astype(BF_NP)),
                "wk": np.ascontiguousarray(wk[:, sl].astype(BF_NP)),
                "wv": np.ascontiguousarray(wv[:, sl].astype(BF_NP)),
                "wo": np.ascontiguousarray(wo[sl, :].astype(BF_NP)),
                "bq": np.ascontiguousarray(np.asarray(bq, np.float32)[sl]),
                "bk": np.ascontiguousarray(np.asarray(bk, np.float32)[sl]),
            }
        )
    return in_maps


def gather_output(results, bo, bv, wo):
    bo = np.asarray(bo, np.float32)
    row = bo + np.asarray(bv, np.float32) @ np.asarray(wo, np.float32)
    out = np.empty((B, N, D), np.float32)
    for b in range(B):
        out[b] = results[2 * b]["out"] + results[2 * b + 1]["out"] + row[None, :]
    return out


def kernel(x, attn_bias, wq, bq, wk, bk, wv, bv, wo, bo, _trace=False):
    nc = _get_nc()
    in_maps = make_in_maps(x, attn_bias, wq, bq, wk, bk, wv, bv, wo, bo)
    res = run_bass_kernel_spmd(nc, in_maps, core_ids=list(range(8)), trace=_trace)
    out = gather_output(res.results, bo, bv, wo)
    if _trace:
        kernel.last_results = res
    return out


# revision 33
# speedup vs baseline: 1.2984x; 1.2551x over previous
"""Biased MHSA Trainium2 kernel (8-core SPMD), bf16 compute.

Sharding: core c -> (batch b = c//2, head-group g = c%2); each core computes
attention for 4 of the 8 heads of one batch and the partial output projection
for those heads. Host sums the two head-group partials per batch and adds
bo + bv @ wo.

Performance structure (steady state is ACT-bound: one EXP [128,1024] per
1.09us, 2 per key-tile iteration):
  - All matmuls in bf16 (PSUM accumulation stays fp32).
  - Bias handled as P = exp(S) * exp(bias): the host precomputes exp(bias)
    in bf16; the multiply runs on DVE in 2x bf16 mode from SBUF.
  - PV matmuls run one key-tile behind the scores so the in-order PE queue
    never waits on the exp->mult chain.
  - Softmax normalization never touches DRAM or the DVE's critical stream:
    1/r via reciprocal_approx_fast on the PSUM aug-row, pu eviction on
    GpSimd, r broadcast across the 64 feature partitions by a tiny f32r PE
    matmul into the PSUM slot pu just vacated, A^T = U^T * bc on DVE. The
    PE/DVE halves of that chain are deferred into the NEXT pass's kt loop
    (kt==1) so pass boundaries stay pipelined; pu tiles are allocated
    lazily at kt==1 to keep the PSUM tag rotation pu(p)->bc(p)->pu(p+1).
  - O-projection tiles for chunk q2-1 are spread through the next pass's
    kt loop (kt 3/6/9/12), evicted on GpSimd, so the psum_mm rotation and
    the DVE stream never see a burst.
  - Startup: every input DMA rides the sync queue in priority order
    (x, wk, wq, wv, wo, biases, then exp(bias) tiles) so projections are
    never starved by the 8MB bias stream; 8 dummy matmuls pre-warm the HAM
    clock gate and a dummy exp preloads the ACT Exp table.
"""

import sys

if "/opt/trn_rl_repo" not in sys.path:
    sys.path.insert(0, "/opt/trn_rl_repo")

from contextlib import ExitStack

import numpy as np
import ml_dtypes

import concourse.bass as bass
from concourse import bacc
import concourse.tile as tile
from concourse import mybir
from concourse.bass_utils import run_bass_kernel_spmd

B, N, D = 4, 2048, 512
H, DH = 8, 64
HG = 4  # heads per core
GD = HG * DH  # 256 features per core
P = 128
QQ = 512  # matmul moving-dim slice
QW = 1024  # q processed in chunks of 1024
NQW = N // QW  # 2
NSL = QW // QQ  # 2
PVLAG = 3  # PV runs 3 key-tiles behind the scores
NKT = N // P  # 16 key tiles
KC = D // P  # 4 contraction chunks for projections
NN = N // QQ  # 4 token chunks for projections
F32 = mybir.dt.float32
F32R = mybir.dt.float32r
BF16 = mybir.dt.bfloat16
BF_NP = ml_dtypes.bfloat16


def build_program():
    nc = bacc.Bacc("TRN2", target_bir_lowering=False)
    xT = nc.dram_tensor("xT", [D, N], BF16, kind="ExternalInput")
    expbT = nc.dram_tensor("expbT", [N, N], BF16, kind="ExternalInput")
    wq = nc.dram_tensor("wq", [D, GD], BF16, kind="ExternalInput")
    wk = nc.dram_tensor("wk", [D, GD], BF16, kind="ExternalInput")
    wv = nc.dram_tensor("wv", [D, GD], BF16, kind="ExternalInput")
    wo = nc.dram_tensor("wo", [GD, D], BF16, kind="ExternalInput")
    bq = nc.dram_tensor("bq", [GD], F32, kind="ExternalInput")
    bk = nc.dram_tensor("bk", [GD], F32, kind="ExternalInput")
    out = nc.dram_tensor("out", [N, D], F32, kind="ExternalOutput")

    with tile.TileContext(nc) as tc, ExitStack() as ctx:
        const = ctx.enter_context(tc.tile_pool(name="const", bufs=1))
        big = ctx.enter_context(tc.tile_pool(name="big", bufs=1))
        p_pool = ctx.enter_context(tc.tile_pool(name="probs", bufs=8))
        small = ctx.enter_context(tc.tile_pool(name="small", bufs=2))
        o_pool = ctx.enter_context(tc.tile_pool(name="outp", bufs=6))
        psum_mm = ctx.enter_context(tc.tile_pool(name="psum_mm", bufs=2, space="PSUM"))
        psum_u = ctx.enter_context(tc.tile_pool(name="psum_u", bufs=1, space="PSUM"))
        dram_p = ctx.enter_context(tc.tile_pool(name="dram_p", bufs=2, space="DRAM"))

        # ---- HAM warm-up + ACT Exp-table preload (runs during input DMAs) --
        warm = const.tile([P, QQ], BF16)
        nc.vector.memset(warm, 0.0)
        expd = const.tile([P, 2], BF16)
        nc.scalar.activation(expd, warm[:, 0:2], mybir.ActivationFunctionType.Exp)
        wps = psum_mm.tile([P, QW], F32, tag="mm", name="warmps")
        for _ in range(40):  # bridge the input-DMA wait so HAM stays warm
            nc.tensor.matmul(wps[:, 0:QQ], warm[:, 0:P], warm, start=True, stop=True)

        # ---- load inputs: one priority-ordered queue (x+weights before the
        # 8MB exp(bias) stream; the DGE spreads descriptors over all 16 HW
        # engines regardless of which queue issues) ----------------------
        bq_s = const.tile([P, 2], F32)
        nc.sync.dma_start(out=bq_s, in_=bq.rearrange("(fc p) -> p fc", p=P))
        bk_s = const.tile([P, 2], F32)
        nc.sync.dma_start(out=bk_s, in_=bk.rearrange("(fc p) -> p fc", p=P))
        bqs = const.tile([P, 2], F32)  # bq * 0.125 (scale folded into Q)
        nc.vector.tensor_scalar_mul(bqs, bq_s, 0.125)
        wq_s = const.tile([P, KC, GD], BF16)
        wk_s = const.tile([P, KC, GD], BF16)
        wv_s = const.tile([P, KC, GD], BF16)
        xT_s = big.tile([P, KC, N], BF16)  # x^T as [128, kc, tok]
        for kc in range(KC):
            nc.sync.dma_start(out=wk_s[:, kc, :], in_=wk[kc * P : (kc + 1) * P, :])
        for nn in range(NN):
            for kc in range(KC):
                nc.sync.dma_start(
                    out=xT_s[:, kc, nn * QQ : (nn + 1) * QQ],
                    in_=xT[kc * P : (kc + 1) * P, nn * QQ : (nn + 1) * QQ],
                )
        for kc in range(KC):
            nc.sync.dma_start(out=wq_s[:, kc, :], in_=wq[kc * P : (kc + 1) * P, :])
        for kc in range(KC):
            nc.sync.dma_start(out=wv_s[:, kc, :], in_=wv[kc * P : (kc + 1) * P, :])
        wo_s = const.tile([P, 2, D], BF16)  # head-pair wo rows: [128, hp, 512]
        nc.sync.dma_start(out=wo_s, in_=wo.rearrange("(g p) d -> p g d", p=P))
        # exp(bias)^T tiles, bf16, after all compute-critical inputs
        ebt = big.tile([P, NQW, NKT, QW], BF16)
        for q2 in range(NQW):
            for kt in range(NKT):
                nc.sync.dma_start(
                    out=ebt[:, q2, kt, :],
                    in_=expbT[kt * P : (kt + 1) * P, q2 * QW : (q2 + 1) * QW],
                )

        # ---- projections (K first: scores need all of K; then Q, then V) --
        # Q^T, K^T: [128, fc, tok] (feature on partitions; head-pair fc has
        # head 2fc on partitions 0..63 and head 2fc+1 on 64..127)
        qT = big.tile([P, 2, N], BF16)
        kT = big.tile([P, 2, N], BF16)
        vaug = big.tile([P, HG, NKT, DH + 1], BF16)  # [tok, h, kt, 64 V | 1]
        nc.vector.memset(vaug[:, :, :, DH : DH + 1], 1.0)
        for w_s, dst, b_ap, scale in (
            (wk_s, kT, bk_s, 1.0),
            (wq_s, qT, bqs, 0.125),
        ):
            for nn in range(NN):
                for fc in range(2):
                    ps = psum_mm.tile([P, QW], F32, tag="mm", name="ps_p")
                    for kc in range(KC):
                        nc.tensor.matmul(
                            ps[:, 0:QQ],
                            (w_s[:, kc, fc * P : (fc + 1) * P]),
                            (xT_s[:, kc, nn * QQ : (nn + 1) * QQ]),
                            start=(kc == 0),
                            stop=(kc == KC - 1),
                        )
                    # (x@w + b) * scale  ==  psum*scale + b*scale
                    nc.vector.tensor_scalar(
                        dst[:, fc, nn * QQ : (nn + 1) * QQ],
                        ps[:, 0:QQ],
                        scale,
                        b_ap[:, fc : fc + 1],
                        op0=mybir.AluOpType.mult,
                        op1=mybir.AluOpType.add,
                    )
        for kt in range(NKT):  # V natural layout (bv folded on host)
            ps = psum_mm.tile([P, QW], F32, tag="mm", name="ps_v")
            for kc in range(KC):
                nc.tensor.matmul(
                    ps[:, 0:GD],
                    (xT_s[:, kc, kt * P : (kt + 1) * P]),
                    (wv_s[:, kc, :]),
                    start=(kc == 0),
                    stop=(kc == KC - 1),
                )
            nc.vector.tensor_copy(
                vaug[:, :, kt, 0:DH],
                ps[:, 0:GD].rearrange("p (h d) -> p h d", h=HG),
            )

        # ---- attention ----
        # A^T (normalized attn out), head pair stacked on 128 partitions:
        # head 2hp+i at partitions i*64..(i+1)*64 of chunk hp. One tile per
        # q chunk so the O-proj tiles for chunk q2-1 carry no (coarse-grain)
        # dependency on chunk q2's normalize writes.
        aTs = [big.tile([P, 2, QW], BF16, name=f"aT{j}") for j in range(NQW)]
        ones64 = const.tile([1, DH], BF16)  # for the 1/r PE broadcast
        nc.vector.memset(ones64, 1.0)
        ident = const.tile([P, P], F32)  # for the PE transpose of r128
        nc.vector.memset(ident, 1.0)
        nc.gpsimd.affine_select(
            ident, ident, pattern=[[1, P]],
            compare_op=mybir.AluOpType.is_equal, fill=0.0,
            base=0, channel_multiplier=-1,
        )

        def norm_stage_a(pu_h):
            # Evict U_aug^T (releases pu's PSUM slot) and launch the r-row
            # partition-fold DMA: [1,1024] row -> DRAM -> [128,8] so the
            # reciprocal is a ~0.2us op instead of a 6.5us one. The DMA
            # latency hides under the next pass's first key-tiles.
            uT = small.tile([DH + 1, QW], F32, tag="uT", name="uT")
            nc.vector.tensor_copy(uT, pu_h)
            r_d = dram_p.tile([QW], F32, tag="rd", name="r_d")
            nc.gpsimd.dma_start(out=r_d[:], in_=uT[DH : DH + 1, :])
            r128 = small.tile([P, QW // P], F32, tag="r128", name="r128")
            nc.gpsimd.dma_start(out=r128, in_=r_d[:].rearrange("(f p) -> p f", p=P))
            return uT, r128

        def norm_stage_a2(i, r128):
            # 1/r on the folded layout; unfold on-chip: one PE transpose
            # [128,8]->[8,128], evict to bf16 rows, then 8 tiny PE
            # broadcasts into the PSUM slot pu vacated. No DRAM round trip.
            nc.vector.reciprocal(r128, r128)
            rrow = psum_u.tile([1, QW], F32, tag=f"u{i}", name="rrow")
            for f in range(QW // P):
                nc.tensor.transpose(
                    rrow[0:1, f * P : (f + 1) * P], r128[:, f : f + 1], ident
                )
            rT = small.tile([1, QW], BF16, tag="rT", name="rT")
            nc.vector.tensor_copy(rT, rrow)
            bc = psum_u.tile([DH, QW], F32, tag=f"u{i}", name="bc")
            for sl in range(NSL):
                ssl = slice(sl * QQ, (sl + 1) * QQ)
                nc.tensor.matmul(bc[:, ssl], ones64, rT[:, ssl], start=True, stop=True)
            return bc

        def norm_stage_b(q2, hp, i, uT, bc):
            nc.vector.tensor_tensor(
                aTs[q2][i * DH : (i + 1) * DH, hp, :],
                uT[0:DH, :],
                bc,
                op=mybir.AluOpType.mult,
            )

        def oproj_tile(t):
            # O[tok, 512] = sum_hp A_pair^T.T @ wo_pair for one token tile
            aT_t = aTs[t // (QW // P)]
            tl = t % (QW // P)
            ps = psum_mm.tile([P, QW], F32, tag="mm", name="ps_o")
            for hp in range(2):
                nc.tensor.matmul(
                    ps[:, 0:D],
                    aT_t[:, hp, tl * P : (tl + 1) * P],
                    wo_s[:, hp, :],
                    start=(hp == 0),
                    stop=(hp == 1),
                )
            ob = o_pool.tile([P, D], F32, name="ob")
            nc.vector.tensor_copy(ob, ps[:, 0:D])
            nc.sync.dma_start(out=out[t * P : (t + 1) * P, :], in_=ob)

        pending_norm = None  # deferred stage-B args from the previous pass
        for q2 in range(NQW):
            for hp in range(2):  # head pair: heads (2hp, 2hp+1) live in fc=hp
                pu = None
                # O-proj tiles for chunk q2-1 spread through this pass
                otiles = (
                    [(q2 - 1) * (QW // P) + hp * 4 + t for t in range(4)]
                    if q2 > 0
                    else []
                )

                def emit_pv(j, sps):
                    for i in range(2):
                        for sl in range(NSL):
                            ssl = slice(sl * QQ, (sl + 1) * QQ)
                            nc.tensor.matmul(
                                pu[i][:, ssl],
                                vaug[:, 2 * hp + i, j, :],
                                sps[i][:, ssl],
                                start=(j == 0),
                                stop=(j == NKT - 1),
                            )

                sp_hist = []
                for kt in range(NKT):
                    # scores for both heads; sl-outer/i-inner so consecutive
                    # instructions pair up on disjoint PE row groups
                    ps = [
                        psum_mm.tile([P, QW], F32, tag="mm", name=f"ps{i}")
                        for i in range(2)
                    ]
                    for sl in range(NSL):
                        ssl = slice(sl * QQ, (sl + 1) * QQ)
                        for i in range(2):
                            ho = i * DH
                            nc.tensor.matmul(
                                ps[i][:, ssl],
                                kT[ho : ho + DH, hp, kt * P : (kt + 1) * P],
                                qT[ho : ho + DH, hp, q2 * QW + sl * QQ :
                                   q2 * QW + (sl + 1) * QQ],
                                start=True,
                                stop=True,
                            )
                    sp_cur = []
                    for i in range(2):
                        sp = p_pool.tile([P, QW], BF16, tag="sp", name="sp")
                        nc.scalar.activation(
                            sp, ps[i], mybir.ActivationFunctionType.Exp
                        )
                        # P = exp(S) * exp(bias): 2x bf16 DVE mode, in place
                        nc.vector.tensor_tensor(
                            sp, sp, ebt[:, q2, kt, :], op=mybir.AluOpType.mult
                        )
                        sp_cur.append(sp)
                    if kt == 3:
                        # previous pass's normalize back half (the r-fold DMA
                        # landed by ~kt2), then this pass's accumulators: the
                        # u-tag rotation is pu(p) -> tr2(p) -> bc(p) -> pu(p+1)
                        if pending_norm:
                            for st in pending_norm:
                                bc = norm_stage_a2(st[2], st[4])
                                norm_stage_b(st[0], st[1], st[2], st[3], bc)
                            pending_norm = None
                        pu = [
                            psum_u.tile([DH + 1, QW], F32, tag=f"u{i}", name=f"pu{i}")
                            for i in range(2)
                        ]
                    if kt in (6, 9, 12, 15) and otiles:
                        oproj_tile(otiles.pop(0))
                    if kt >= PVLAG:
                        emit_pv(kt - PVLAG, sp_hist[0])
                        sp_hist.pop(0)
                    sp_hist.append(sp_cur)
                for j in range(NKT - PVLAG, NKT):
                    emit_pv(j, sp_hist[0])
                    sp_hist.pop(0)
                while otiles:
                    oproj_tile(otiles.pop(0))
                pending_norm = []
                for i in range(2):
                    uT, r128 = norm_stage_a(pu[i])
                    pending_norm.append([q2, hp, i, uT, r128])
        # final pass: nothing left to overlap with; emit the rest directly
        for st in pending_norm:
            bc = norm_stage_a2(st[2], st[4])
            norm_stage_b(st[0], st[1], st[2], st[3], bc)
        for t in range((NQW - 1) * QW // P, NQW * QW // P):
            oproj_tile(t)

    nc.compile()
    return nc


_NC = None


def _get_nc():
    global _NC
    if _NC is None:
        _NC = build_program()
    return _NC


def make_in_maps(x, attn_bias, wq, bq, wk, bk, wv, bv, wo, bo):
    x = np.asarray(x, np.float32)
    attn_bias = np.asarray(attn_bias, np.float32)
    expbT_b = [
        np.ascontiguousarray(np.exp(attn_bias[b, 0]).T.astype(BF_NP))
        for b in range(B)
    ]
    xT_b = [np.ascontiguousarray(x[b].T.astype(BF_NP)) for b in range(B)]
    wq = np.asarray(wq, np.float32)
    wk = np.asarray(wk, np.float32)
    wv = np.asarray(wv, np.float32)
    wo = np.asarray(wo, np.float32)
    in_maps = []
    for c in range(8):
        b, g = c // 2, c % 2
        sl = slice(g * GD, (g + 1) * GD)
        in_maps.append(
            {
                "xT": xT_b[b],
                "expbT": expbT_b[b],
                "wq": np.ascontiguousarray(wq[:, sl].astype(BF_NP)),
                "wk": np.ascontiguousarray(wk[:, sl].astype(BF_NP)),
                "wv": np.ascontiguousarray(wv[:, sl].astype(BF_NP)),
                "wo": np.ascontiguousarray(wo[sl, :].astype(BF_NP)),
                "bq": np.ascontiguousarray(np.asarray(bq, np.float32)[sl]),
                "bk": np.ascontiguousarray(np.asarray(bk, np.float32)[sl]),
            }
        )
    return in_maps


def gather_output(results, bo, bv, wo):
    bo = np.asarray(bo, np.float32)
    row = bo + np.asarray(bv, np.float32) @ np.asarray(wo, np.float32)
    out = np.empty((B, N, D), np.float32)
    for b in range(B):
        out[b] = results[2 * b]["out"] + results[2 * b + 1]["out"] + row[None, :]
    return out


def kernel(x, attn_bias, wq, bq, wk, bk, wv, bv, wo, bo, _trace=False):
    nc = _get_nc()
    in_maps = make_in_maps(x, attn_bias, wq, bq, wk, bk, wv, bv, wo, bo)
    res = run_bass_kernel_spmd(nc, in_maps, core_ids=list(range(8)), trace=_trace)
    out = gather_output(res.results, bo, bv, wo)
    if _trace:
        kernel.last_results = res
    return out
